# revision 9
# baseline (speedup 1.0000x reference)
"""nn_BinaryMoSLinear Trainium2 kernel: 8 NeuronCores, data-parallel over tokens.

kernel(**inputs) takes the FULL reference.setup_inputs() tensors and returns
the FULL [4, 2048, 4096] f32 output. Tokens are sharded 1024/core; weight,
bias, gate and channel scales are replicated (no collectives).

v5: partial fp8 DoubleRow + dispatch diet. The first NBF h-chunks run as
bf16-stationary x fp8-moving matmuls (1 col/cycle); the last 2*NDR chunks run
as fp8xfp8 DoubleRow matmuls (2 K-rows/cycle, 2x flops) with x_s written
straight to fp8 by the in_scale multiply. NDR=7 (f=0.44) lands ~1.6e-2 vs the
2e-2 gate (CPU model matches hw bit-for-bit on the fp8 part). out_scale is
computed on the PE (r.T @ ocs, K=4) instead of 4 DVE passes over broadcast
ocs tiles, which kills the 16.8MB ocsb stream and the DVE chains that stalled
window boundaries. W streams in 3 grouped DMAs per oc (6 chunks each) plus
one DoubleRow slab per oc; bias loads once per oc (shared by both gens).
"""
from contextlib import ExitStack

import concourse.bass as bass
import concourse.mybir as mybir

F32 = mybir.dt.float32
BF16 = mybir.dt.bfloat16
FP8 = mybir.dt.float8e4
AF = mybir.ActivationFunctionType
OP = mybir.AluOpType
DR = mybir.MatmulPerfMode.DoubleRow


def build_moe8(ctx, tc, outs, ins, cfg):
    nc = tc.nc
    H, O, Nc, E = cfg["H"], cfg["O"], cfg["Nc"], cfg["E"]
    ow = cfg["ow"]
    NDR = cfg["NDR"]         # fp8 DoubleRow chunk-pairs at the tail of H
    GS = cfg["GS"]           # bf16 w-chunks fetched per DMA dispatch
    NH = H // 128            # 32 h-chunks
    NBF = NH - 2 * NDR       # bf16 h-chunks at the head of H
    NG = (NBF + GS - 1) // GS
    OC = O // ow             # 8 output column blocks
    y = outs["y"]

    pool = ctx.enter_context(tc.tile_pool(name="sb", bufs=1))
    ctx.enter_context(nc.allow_low_precision(reason="bf16/fp8 pipeline"))
    psum = ctx.enter_context(tc.tile_pool(name="ps", bufs=1, space="PSUM"))

    # ---- constants (tiles declared here, DMAs woven into the x stream) ----
    gwp = pool.tile([128, NH * E], BF16, name="gwp", tag="gwp", bufs=1)
    ics_t = pool.tile([E, H], BF16, name="ics", tag="ics", bufs=1)
    ocs_sb = pool.tile([E, O], BF16, name="ocs_sb", tag="ocs", bufs=1)
    ones_e1 = pool.tile([E, 1], BF16, name="ones_e1", tag="ones_e1", bufs=1)
    ones_1e = pool.tile([1, E], BF16, name="ones_1e", tag="ones_1e", bufs=1)

    def dma_consts(h):
        if h == 0:
            nc.scalar.dma_start(gwp[:], ins["gwP"][:, :])
        elif h == 2:
            nc.scalar.dma_start(ics_t[:], ins["ics"][:, :])
        elif h == 4:
            nc.sync.dma_start(ones_e1[:], ins["ones_e"][:, 0:1])
            nc.sync.dma_start(ones_1e[:], ins["ones_e"][0:1, :])
        elif h == 6:
            nc.scalar.dma_start(ocs_sb[:], ins["ocs_f"][:, :])

    xh = {}      # (half, hp) -> [128, 1024] bf16 x tile (chunk pair)
    x8 = {}      # (half, hp) -> [128, 2, 512] fp8 x_s tile (DoubleRow pairs)
    rts = {}     # half -> [E, 512] bf16 routing weights
    wst = {}     # (gen, oc, g) -> [128, GS, ow] fp8 weight group (bf16 path)
    w8d = {}     # (gen, oc) -> [128, 2*NDR, 512] fp8 DoubleRow slab
    logits = {}
    bias_t = {}
    os_t = {}
    mains = {}

    def dma_x(hf, hp):
        # one dispatch per h-chunk PAIR: halves DMA dispatch serialization
        t = pool.tile([128, 1024], BF16, name=f"x{hf}_{hp}", tag=f"xh{hf}",
                      bufs=NH // 2)
        eng = nc.sync if hp % 2 == 0 else nc.scalar
        eng.dma_start(t[:], ins[f"xP{hf}"][:, hp * 1024:(hp + 1) * 1024])
        xh[(hf, hp)] = t

    def xsl(hf, h, j0, j1):
        return xh[(hf, h // 2)][:, (h % 2) * 512 + j0:(h % 2) * 512 + j1]

    def dma_w(gen, oc, g):
        gs = min(GS, NBF - g * GS)
        t = pool.tile([128, gs, ow], FP8, name=f"w{gen}_{oc}_{g}", tag="wst",
                      bufs=2 * NG)
        eng = nc.sync if g % 2 == 0 else nc.scalar
        eng.dma_start(t[:, :, :], ins["wbP"][:, g * GS:g * GS + gs,
                                             oc * ow:(oc + 1) * ow])
        wst[(gen, oc, g)] = t

    def dma_w8(gen, oc):
        if NDR == 0:
            return
        t = pool.tile([128, 2 * NDR, 512], FP8, name=f"wd{gen}_{oc}",
                      tag="w8d", bufs=2)
        nc.sync.dma_start(t[:, :, :], ins["wdr"][:, oc, :])
        w8d[(gen, oc)] = t

    def dma_bias(oc):
        if oc in bias_t:
            return
        b = pool.tile([128, ow], F32, name=f"bias_{oc}", tag="bias", bufs=OC)
        nc.scalar.dma_start(b[:], ins["bias2"][:, oc * ow:(oc + 1) * ow])
        bias_t[oc] = b

    # ---- per-half router pieces ----
    def router_mm(hf, h):
        if hf not in logits:
            logits[hf] = psum.tile([E, 512], F32, name=f"logits{hf}",
                                   tag="plog", bufs=1)
        nc.tensor.matmul(logits[hf][:], gwp[:, h * E:(h + 1) * E],
                         xsl(hf, h, 0, 512), start=(h == 0),
                         stop=(h == NH - 1))

    exs = {}
    rcps = {}

    def softmax_a(hf):
        ex = pool.tile([E, 512], BF16, name=f"ex{hf}", tag="ex", bufs=2)
        nc.scalar.activation(ex[:], logits.pop(hf)[:], AF.Exp)
        exs[hf] = ex
        ssum = psum.tile([1, 512], F32, name=f"ssum{hf}", tag="psm", bufs=3)
        nc.tensor.matmul(ssum[:], ones_e1[:], ex[:], start=True, stop=True)
        rcp32 = pool.tile([1, 512], F32, name=f"rcp32{hf}", tag="rcp32",
                          bufs=2)
        nc.vector.reciprocal_approx_fast(rcp32[:], ssum[:])
        rcpb = pool.tile([1, 512], BF16, name=f"rcpb{hf}", tag="rcpb", bufs=2)
        nc.vector.tensor_copy(rcpb[:], rcp32[:])
        rcps[hf] = rcpb

    def softmax_b(hf):
        bc = psum.tile([E, 512], F32, name=f"bc{hf}", tag="psm", bufs=3)
        nc.tensor.matmul(bc[:], ones_1e[:], rcps.pop(hf)[:], start=True,
                         stop=True)
        rt = pool.tile([E, 512], BF16, name=f"rt{hf}", tag="rt", bufs=2)
        nc.vector.tensor_tensor(rt[:], exs.pop(hf)[:], bc[:], OP.mult)
        rts[hf] = rt

    def in_scale(hf, h):
        # x_s = x * (r @ ics); bf16 chunks in place over the x tile, fp8
        # (DoubleRow) chunks written straight into the fp8 pair tile
        isp = psum.tile([128, 512], F32, name=f"isp{hf}_{h}", tag="psm",
                        bufs=3)
        nc.tensor.matmul(isp[:], ics_t[:, h * 128:(h + 1) * 128],
                         rts[hf][:], start=True, stop=True)
        xt = xsl(hf, h, 0, 512)
        if h >= NBF:
            key = (hf, h // 2)
            if key not in x8:
                x8[key] = pool.tile([128, 2, 512], FP8,
                                    name=f"x8_{hf}_{h // 2}", tag=f"x8{hf}",
                                    bufs=max(NDR, 1))
            nc.vector.tensor_tensor(x8[key][:, h % 2, :], xt, isp[:], OP.mult)
        else:
            nc.vector.tensor_tensor(xt, xt, isp[:], OP.mult)

    # ---- main-phase matmul emitters ----
    def mm_bf(gen, oc, tcc, h):
        nc.tensor.matmul(mains[(oc, tcc)][:],
                         xsl(gen, h, (tcc % 4) * 128, (tcc % 4 + 1) * 128),
                         wst[(gen, oc, h // GS)][:, h % GS, :],
                         start=(h == 0), stop=(NDR == 0 and h == NBF - 1))

    def mm_dr(gen, oc, tcc, ihp):
        hp = NBF // 2 + ihp
        xs = x8[(gen, hp)][:, :, (tcc % 4) * 128:(tcc % 4 + 1) * 128]
        wt = w8d[(gen, oc)]
        for half in range(2):
            nc.tensor.matmul(
                mains[(oc, tcc)][:, half * 256:(half + 1) * 256], xs,
                wt[:, 2 * ihp:2 * ihp + 2, half * 256:(half + 1) * 256],
                start=False, stop=(ihp == NDR - 1), perf_mode=DR)

    # ---- main-phase window pieces ----
    def emit_os(gen, oc, tcs):
        # out_scale on the PE: os[tok, ocol] = r[:, tok].T @ ocs[:, ocol]
        for tcc in tcs:
            osp = psum.tile([128, ow], F32, name=f"osp{gen}_{oc}_{tcc}",
                            tag="psm", bufs=3)
            nc.tensor.matmul(osp[:],
                             rts[gen][:, (tcc % 4) * 128:(tcc % 4 + 1) * 128],
                             ocs_sb[:, oc * ow:(oc + 1) * ow],
                             start=True, stop=True)
            t = pool.tile([128, ow], F32, name=f"os{gen}_{oc}_{tcc}",
                          tag="os", bufs=8)
            nc.vector.tensor_copy(t[:], osp[:])
            os_t[(oc, tcc)] = t

    def emit_drain(gen, oc, tcc):
        yt = pool.tile([128, ow], F32, name=f"yt{gen}_{oc}_{tcc}", tag="yt",
                       bufs=6)
        nc.vector.tensor_tensor(yt[:], mains.pop((oc, tcc))[:],
                                os_t.pop((oc, tcc))[:], OP.mult)
        yt2 = pool.tile([128, ow], F32, name=f"yt2{gen}_{oc}_{tcc}",
                        tag="yt2", bufs=6)
        nc.gpsimd.tensor_tensor(yt2[:], yt[:], bias_t[oc][:], OP.add)
        nc.scalar.dma_start(y[tcc * 128:(tcc + 1) * 128,
                              oc * ow:(oc + 1) * ow], yt2[:])

    # ---- prologue: half A router + softmax; in_scale A fused into w0 ----
    for h in range(NH):
        if h % 2 == 0:
            dma_x(0, h // 2)
        dma_consts(h)
        router_mm(0, h)
    softmax_a(0)
    softmax_b(0)
    dma_bias(0)
    for g in range(NG):
        dma_w(0, 0, g)
    dma_w8(0, 0)
    in_scale(0, 0)
    in_scale(0, 1)
    emit_os(0, 0, [0, 1])
    emit_os(0, 0, [2, 3])

    # ---- main: windows (gen, oc, pair); gen 0 = half A (tc 0-3),
    #      gen 1 = half B (tc 4-7). Window = 2 PSUM banks, ring of 4.
    #      Each window runs NBF bf16 steps then 2*NDR DoubleRow steps
    #      (pair-major: one (pair, tc) per step) = NH steps total. ----
    windows = [(g, oc, p) for g in range(2) for oc in range(OC)
               for p in range(2)]

    # extra ops (half-B prologue) interleaved into half-A windows, per h step
    extras = {}                      # (widx, h) -> list of callables
    for hp in range(NH // 2):        # window 0: stream x half B,
        extras.setdefault((0, hp * 2), []).append(
            lambda hp=hp: dma_x(1, hp))
    for h in range(NH - 2):          # ... and in_scale A two steps ahead
        extras.setdefault((0, h), []).append(lambda h=h: in_scale(0, h + 2))
    for h2 in range(NH):             # router B fills window-0 JIT stalls
        # (x-B pair h2//2 lands ~2 steps before step h2+5); overflow -> w1
        w, s = (0, h2 + 5) if h2 + 5 < NH else (1, 2 * (h2 + 5 - NH))
        extras.setdefault((w, s), []).append(lambda h2=h2: router_mm(1, h2))
    extras.setdefault((1, 12), []).append(lambda: softmax_a(1))
    extras.setdefault((1, 24), []).append(lambda: softmax_b(1))
    for h2 in range(NH):             # windows 2-12: in_scale B (3 / window)
        w = 2 + h2 // 3
        extras.setdefault((w, (h2 % 3) * 10 + 5), []).append(
            lambda h2=h2: in_scale(1, h2))

    def fire_extras(widx, step):
        for fn in extras.pop((widx, step), ()):
            fn()

    prev = None
    for widx, (gen, oc, p) in enumerate(windows):
        tcs = [gen * 4 + p * 2, gen * 4 + p * 2 + 1]
        last = widx == len(windows) - 1
        if prev is not None:
            for tcc in prev[1]:
                emit_drain(prev[0], prev[2], tcc)
        if widx >= 2:
            emit_os(gen, oc, tcs)
        for tcc in tcs:
            mains[(oc, tcc)] = psum.tile([128, ow], F32,
                                         name=f"mp{gen}_{oc}_{tcc}",
                                         tag="mps", bufs=4)
        ngen, noc = (gen, oc + 1) if oc + 1 < OC else (gen + 1, 0)
        pref = p == 1 and ngen < 2
        if pref:
            dma_bias(noc)
        if last:
            # column-major: finish tc by tc so the tail drain is short
            for tcc in tcs:
                for h in range(NBF):
                    mm_bf(gen, oc, tcc, h)
                for ihp in range(NDR):
                    mm_dr(gen, oc, tcc, ihp)
                if tcc != tcs[1]:
                    emit_drain(gen, oc, tcc)
            # final tc drained in halves so DVE/GpSimd/DMA pipeline
            tcc = tcs[1]
            mp = mains.pop((oc, tcc))
            ot = os_t.pop((oc, tcc))
            for k in range(2):
                sl = slice(k * (ow // 2), (k + 1) * (ow // 2))
                yt = pool.tile([128, ow // 2], F32, name=f"ytf{k}", tag="ytf",
                               bufs=2)
                nc.vector.tensor_tensor(yt[:], mp[:, sl], ot[:, sl], OP.mult)
                yt2 = pool.tile([128, ow // 2], F32, name=f"ytf2{k}",
                                tag="ytf2", bufs=2)
                nc.vector.tensor_tensor(yt2[:], yt[:],
                                        bias_t[oc][:, sl], OP.add)
                nc.scalar.dma_start(y[tcc * 128:(tcc + 1) * 128,
                                      oc * ow + k * (ow // 2):
                                      oc * ow + (k + 1) * (ow // 2)], yt2[:])
            prev = None
            break
        for h in range(NBF):
            fire_extras(widx, h)
            if pref and h % GS == 0:
                dma_w(ngen, noc, h // GS)
            for tcc in tcs:
                mm_bf(gen, oc, tcc, h)
        for step in range(NBF, NH):
            fire_extras(widx, step)
            s = step - NBF
            if pref and s == 0:
                dma_w8(ngen, noc)
            # pair-major: window 0 consumes pair hp at step NBF+s, one step
            # after its in_scale fires (JIT)
            ihp = s // 2
            tcc = tcs[s % 2]
            mm_dr(gen, oc, tcc, ihp)
        prev = (gen, tcs, oc)
    assert not extras, f"unconsumed extras: {list(extras)[:4]}"


import numpy as np
import ml_dtypes

NCORES = 8
B, S, H, O, E = 4, 2048, 4096, 4096, 4
N = B * S
Nc = N // NCORES
NDR = 7
NBF = H // 128 - 2 * NDR
GS = 9
CFG = dict(H=H, O=O, Nc=Nc, E=E, ow=512, NDR=NDR, GS=GS)
BF16_NP = ml_dtypes.bfloat16
FP8_NP = ml_dtypes.float8_e4m3

TRACE = False
LAST_EXEC_NS = None
LAST_TRACE_PATH = None
_NC_CACHE = None


def _get_nc():
    global _NC_CACHE
    if _NC_CACHE is None:
        import concourse.bacc as bacc
        import concourse.tile as tile
        nc = bacc.Bacc("TRN2", target_bir_lowering=False, debug=False,
                       num_devices=NCORES)
        ins_aps = {
            "xP0": nc.dram_tensor("xP0", [128, (H // 128) * 512], BF16,
                                  kind="ExternalInput").ap(),
            "xP1": nc.dram_tensor("xP1", [128, (H // 128) * 512], BF16,
                                  kind="ExternalInput").ap(),
            "wbP": nc.dram_tensor("wbP", [128, NBF, O], FP8,
                                  kind="ExternalInput").ap(),
            "wdr": nc.dram_tensor("wdr", [128, O // 512, NDR * 2 * 512],
                                  FP8, kind="ExternalInput").ap(),
            "gwP": nc.dram_tensor("gwP", [128, (H // 128) * E], BF16,
                                  kind="ExternalInput").ap(),
            "ics": nc.dram_tensor("ics", [E, H], BF16, kind="ExternalInput").ap(),
            "ocs_f": nc.dram_tensor("ocs_f", [E, O], BF16,
                                    kind="ExternalInput").ap(),
            "bias2": nc.dram_tensor("bias2", [128, O], F32,
                                    kind="ExternalInput").ap(),
            "ones_e": nc.dram_tensor("ones_e", [E, E], BF16,
                                     kind="ExternalInput").ap(),
        }
        outs_aps = {"y": nc.dram_tensor("y", [Nc, O], F32,
                                        kind="ExternalOutput").ap()}
        with tile.TileContext(nc) as tc:
            with ExitStack() as ctx:
                build_moe8(ctx, tc, outs_aps, ins_aps, CFG)
        nc.compile()
        _NC_CACHE = nc
    return _NC_CACHE


def kernel(x, weight, bias, gate_w, in_channel_scale, out_channel_scale):
    """Full inputs in, full output out; distributes over 8 NeuronCores."""
    global LAST_EXEC_NS, LAST_TRACE_PATH
    from concourse.bass_utils import run_bass_kernel_spmd

    x = np.asarray(x, dtype=np.float32)
    weight = np.asarray(weight, dtype=np.float32)
    bias = np.asarray(bias, dtype=np.float32)
    gate_w = np.asarray(gate_w, dtype=np.float32)
    ics = np.asarray(in_channel_scale, dtype=np.float32)
    ocs = np.asarray(out_channel_scale, dtype=np.float32)

    nc = _get_nc()
    xf = x.reshape(N, H)
    wsT = np.sign(weight).T.astype(FP8_NP)          # [H, O]
    # bf16-path weights: [p, h, o] so one DMA grabs GS chunks x 512 ocols
    wbP = np.ascontiguousarray(
        wsT[:NBF * 128, :].reshape(NBF, 128, O).transpose(1, 0, 2))
    # DoubleRow tail: [p, oc, (ihp, plane), (half, c)]
    wdr = np.ascontiguousarray(
        wsT[NBF * 128:, :]
        .reshape(NDR, 2, 128, O // 512, 2, 256)      # ihp plane p oc half c
        .transpose(2, 3, 0, 1, 4, 5)
        .reshape(128, O // 512, NDR * 2 * 512))
    gwP = np.ascontiguousarray(
        gate_w.T.reshape(H // 128, 128, E).transpose(1, 0, 2)
        .reshape(128, (H // 128) * E)).astype(BF16_NP)
    ics_b = ics.astype(BF16_NP)
    ocs_b = ocs.astype(BF16_NP)
    bias2 = np.ascontiguousarray(np.broadcast_to(bias[None, :], (128, O)))
    ones_e = np.ones((E, E), dtype=BF16_NP)
    in_maps = []
    for c in range(NCORES):
        xt_c = xf[c * Nc:(c + 1) * Nc, :].T.reshape(H // 128, 128, Nc)
        xp = [np.ascontiguousarray(
                  xt_c[:, :, hf * 512:(hf + 1) * 512].transpose(1, 0, 2)
                  .reshape(128, (H // 128) * 512)).astype(BF16_NP)
              for hf in range(2)]
        in_maps.append({
            "xP0": xp[0], "xP1": xp[1],
            "wbP": wbP, "wdr": wdr, "gwP": gwP, "ics": ics_b,
            "ocs_f": ocs_b, "bias2": bias2, "ones_e": ones_e,
        })
    res = run_bass_kernel_spmd(nc, in_maps, core_ids=list(range(NCORES)),
                               trace=TRACE)
    if TRACE:
        LAST_EXEC_NS = res.exec_time_ns
        if res.instructions_and_trace:
            LAST_TRACE_PATH = res.instructions_and_trace[1]
    yfull = np.concatenate([res.results[c]["y"] for c in range(NCORES)], axis=0)
    return yfull.reshape(B, S, O)


# revision 11
# speedup vs baseline: 1.0012x; 1.0012x over previous
"""nn_BinaryMoSLinear Trainium2 kernel: 8 NeuronCores, data-parallel over tokens.

kernel(**inputs) takes the FULL reference.setup_inputs() tensors and returns
the FULL [4, 2048, 4096] f32 output. Tokens are sharded 1024/core; weight,
bias, gate and channel scales are replicated (no collectives).

v5: partial fp8 DoubleRow + dispatch diet. The first NBF h-chunks run as
bf16-stationary x fp8-moving matmuls (1 col/cycle); the last 2*NDR chunks run
as fp8xfp8 DoubleRow matmuls (2 K-rows/cycle, 2x flops) with x_s written
straight to fp8 by the in_scale multiply. NDR=7 (f=0.44) lands ~1.6e-2 vs the
2e-2 gate (CPU model matches hw bit-for-bit on the fp8 part). out_scale is
computed on the PE (r.T @ ocs, K=4) instead of 4 DVE passes over broadcast
ocs tiles, which kills the 16.8MB ocsb stream and the DVE chains that stalled
window boundaries. W streams in 3 grouped DMAs per oc (6 chunks each) plus
one DoubleRow slab per oc; bias loads once per oc (shared by both gens).
"""
from contextlib import ExitStack

import concourse.bass as bass
import concourse.mybir as mybir

F32 = mybir.dt.float32
BF16 = mybir.dt.bfloat16
FP8 = mybir.dt.float8e4
AF = mybir.ActivationFunctionType
OP = mybir.AluOpType
DR = mybir.MatmulPerfMode.DoubleRow


def build_moe8(ctx, tc, outs, ins, cfg):
    nc = tc.nc
    H, O, Nc, E = cfg["H"], cfg["O"], cfg["Nc"], cfg["E"]
    ow = cfg["ow"]
    NDR = cfg["NDR"]         # fp8 DoubleRow chunk-pairs at the tail of H
    GS = cfg["GS"]           # bf16 w-chunks fetched per DMA dispatch
    NH = H // 128            # 32 h-chunks
    NBF = NH - 2 * NDR       # bf16 h-chunks at the head of H
    NG = (NBF + GS - 1) // GS
    OC = O // ow             # 8 output column blocks
    y = outs["y"]

    pool = ctx.enter_context(tc.tile_pool(name="sb", bufs=1))
    ctx.enter_context(nc.allow_low_precision(reason="bf16/fp8 pipeline"))
    psum = ctx.enter_context(tc.tile_pool(name="ps", bufs=1, space="PSUM"))

    # ---- constants (tiles declared here, DMAs woven into the x stream) ----
    gwp = pool.tile([128, NH * E], BF16, name="gwp", tag="gwp", bufs=1)
    ics_t = pool.tile([E, H], BF16, name="ics", tag="ics", bufs=1)
    ocs_sb = pool.tile([E, O], BF16, name="ocs_sb", tag="ocs", bufs=1)
    ones_e1 = pool.tile([E, 1], BF16, name="ones_e1", tag="ones_e1", bufs=1)
    ones_1e = pool.tile([1, E], BF16, name="ones_1e", tag="ones_1e", bufs=1)

    def dma_consts(h):
        if h == 0:
            nc.scalar.dma_start(gwp[:], ins["gwP"][:, :])
        elif h == 2:
            nc.scalar.dma_start(ics_t[:], ins["ics"][:, :])
        elif h == 4:
            nc.sync.dma_start(ones_e1[:], ins["ones_e"][:, 0:1])
            nc.sync.dma_start(ones_1e[:], ins["ones_e"][0:1, :])
        elif h == 6:
            nc.scalar.dma_start(ocs_sb[:], ins["ocs_f"][:, :])

    xh = {}      # (half, hp) -> [128, 1024] bf16 x tile (chunk pair)
    x8 = {}      # (half, hp) -> [128, 2, 512] fp8 x_s tile (DoubleRow pairs)
    rts = {}     # half -> [E, 512] bf16 routing weights
    wst = {}     # (gen, oc, g) -> [128, GS, ow] fp8 weight group (bf16 path)
    w8d = {}     # (gen, oc) -> [128, 2*NDR, 512] fp8 DoubleRow slab
    logits = {}
    bias_t = {}
    os_t = {}
    mains = {}

    def dma_x(hf, hp):
        # one dispatch per h-chunk PAIR: halves DMA dispatch serialization
        t = pool.tile([128, 1024], BF16, name=f"x{hf}_{hp}", tag=f"xh{hf}",
                      bufs=NH // 2)
        eng = nc.sync if hp % 2 == 0 else nc.scalar
        eng.dma_start(t[:], ins[f"xP{hf}"][:, hp * 1024:(hp + 1) * 1024])
        xh[(hf, hp)] = t

    def xsl(hf, h, j0, j1):
        return xh[(hf, h // 2)][:, (h % 2) * 512 + j0:(h % 2) * 512 + j1]

    def dma_w(gen, oc, g):
        gs = min(GS, NBF - g * GS)
        t = pool.tile([128, gs, ow], FP8, name=f"w{gen}_{oc}_{g}", tag="wst",
                      bufs=2 * NG)
        eng = nc.sync if g % 2 == 0 else nc.scalar
        eng.dma_start(t[:, :, :], ins["wbP"][:, g * GS:g * GS + gs,
                                             oc * ow:(oc + 1) * ow])
        wst[(gen, oc, g)] = t

    def dma_w8(gen, oc):
        if NDR == 0:
            return
        t = pool.tile([128, 2 * NDR, 512], FP8, name=f"wd{gen}_{oc}",
                      tag="w8d", bufs=2)
        nc.sync.dma_start(t[:, :, :], ins["wdr"][:, oc, :])
        w8d[(gen, oc)] = t

    def dma_bias(oc):
        if oc in bias_t:
            return
        b = pool.tile([128, ow], F32, name=f"bias_{oc}", tag="bias", bufs=OC)
        nc.scalar.dma_start(b[:], ins["bias2"][:, oc * ow:(oc + 1) * ow])
        bias_t[oc] = b

    # ---- per-half router pieces ----
    def router_mm(hf, h):
        if hf not in logits:
            logits[hf] = psum.tile([E, 512], F32, name=f"logits{hf}",
                                   tag="plog", bufs=1)
        nc.tensor.matmul(logits[hf][:], gwp[:, h * E:(h + 1) * E],
                         xsl(hf, h, 0, 512), start=(h == 0),
                         stop=(h == NH - 1))

    exs = {}
    rcps = {}

    def softmax_a(hf):
        ex = pool.tile([E, 512], BF16, name=f"ex{hf}", tag="ex", bufs=2)
        nc.scalar.activation(ex[:], logits.pop(hf)[:], AF.Exp)
        exs[hf] = ex
        ssum = psum.tile([1, 512], F32, name=f"ssum{hf}", tag="psm", bufs=3)
        nc.tensor.matmul(ssum[:], ones_e1[:], ex[:], start=True, stop=True)
        rcp32 = pool.tile([1, 512], F32, name=f"rcp32{hf}", tag="rcp32",
                          bufs=2)
        nc.vector.reciprocal_approx_fast(rcp32[:], ssum[:])
        rcpb = pool.tile([1, 512], BF16, name=f"rcpb{hf}", tag="rcpb", bufs=2)
        nc.vector.tensor_copy(rcpb[:], rcp32[:])
        rcps[hf] = rcpb

    def softmax_b(hf):
        bc = psum.tile([E, 512], F32, name=f"bc{hf}", tag="psm", bufs=3)
        nc.tensor.matmul(bc[:], ones_1e[:], rcps.pop(hf)[:], start=True,
                         stop=True)
        rt = pool.tile([E, 512], BF16, name=f"rt{hf}", tag="rt", bufs=2)
        nc.vector.tensor_tensor(rt[:], exs.pop(hf)[:], bc[:], OP.mult)
        rts[hf] = rt

    def in_scale(hf, h):
        # x_s = x * (r @ ics); bf16 chunks in place over the x tile, fp8
        # (DoubleRow) chunks written straight into the fp8 pair tile
        isp = psum.tile([128, 512], F32, name=f"isp{hf}_{h}", tag="psm",
                        bufs=3)
        nc.tensor.matmul(isp[:], ics_t[:, h * 128:(h + 1) * 128],
                         rts[hf][:], start=True, stop=True)
        xt = xsl(hf, h, 0, 512)
        if h >= NBF:
            key = (hf, h // 2)
            if key not in x8:
                x8[key] = pool.tile([128, 2, 512], FP8,
                                    name=f"x8_{hf}_{h // 2}", tag=f"x8{hf}",
                                    bufs=max(NDR, 1))
            nc.vector.tensor_tensor(x8[key][:, h % 2, :], xt, isp[:], OP.mult)
        else:
            nc.vector.tensor_tensor(xt, xt, isp[:], OP.mult)

    # ---- main-phase matmul emitters ----
    def mm_bf(gen, oc, tcc, h):
        nc.tensor.matmul(mains[(oc, tcc)][:],
                         xsl(gen, h, (tcc % 4) * 128, (tcc % 4 + 1) * 128),
                         wst[(gen, oc, h // GS)][:, h % GS, :],
                         start=(h == 0), stop=(NDR == 0 and h == NBF - 1))

    def mm_dr(gen, oc, tcc, ihp):
        hp = NBF // 2 + ihp
        xs = x8[(gen, hp)][:, :, (tcc % 4) * 128:(tcc % 4 + 1) * 128]
        wt = w8d[(gen, oc)]
        for half in range(2):
            nc.tensor.matmul(
                mains[(oc, tcc)][:, half * 256:(half + 1) * 256], xs,
                wt[:, 2 * ihp:2 * ihp + 2, half * 256:(half + 1) * 256],
                start=False, stop=(ihp == NDR - 1), perf_mode=DR)

    # ---- main-phase window pieces ----
    def emit_os(gen, oc, tcs):
        # out_scale on the PE: os[tok, ocol] = r[:, tok].T @ ocs[:, ocol]
        for tcc in tcs:
            osp = psum.tile([128, ow], F32, name=f"osp{gen}_{oc}_{tcc}",
                            tag="psm", bufs=3)
            nc.tensor.matmul(osp[:],
                             rts[gen][:, (tcc % 4) * 128:(tcc % 4 + 1) * 128],
                             ocs_sb[:, oc * ow:(oc + 1) * ow],
                             start=True, stop=True)
            t = pool.tile([128, ow], F32, name=f"os{gen}_{oc}_{tcc}",
                          tag="os", bufs=8)
            nc.vector.tensor_copy(t[:], osp[:])
            os_t[(oc, tcc)] = t

    def emit_drain(gen, oc, tcc):
        yt = pool.tile([128, ow], F32, name=f"yt{gen}_{oc}_{tcc}", tag="yt",
                       bufs=6)
        nc.vector.tensor_tensor(yt[:], mains.pop((oc, tcc))[:],
                                os_t.pop((oc, tcc))[:], OP.mult)
        yt2 = pool.tile([128, ow], F32, name=f"yt2{gen}_{oc}_{tcc}",
                        tag="yt2", bufs=6)
        nc.gpsimd.tensor_tensor(yt2[:], yt[:], bias_t[oc][:], OP.add)
        nc.scalar.dma_start(y[tcc * 128:(tcc + 1) * 128,
                              oc * ow:(oc + 1) * ow], yt2[:])

    # ---- prologue: half A router + softmax; in_scale A fused into w0 ----
    for h in range(NH):
        if h % 2 == 0:
            dma_x(0, h // 2)
        dma_consts(h)
        router_mm(0, h)
    softmax_a(0)
    softmax_b(0)
    dma_bias(0)
    for g in range(NG):
        dma_w(0, 0, g)
    dma_w8(0, 0)
    for hp in range(NH // 2):   # x half B behind w-oc0 in queue order
        dma_x(1, hp)
    in_scale(0, 0)
    in_scale(0, 1)
    emit_os(0, 0, [0, 1])
    emit_os(0, 0, [2, 3])

    # ---- main: windows (gen, oc, pair); gen 0 = half A (tc 0-3),
    #      gen 1 = half B (tc 4-7). Window = 2 PSUM banks, ring of 4.
    #      Each window runs NBF bf16 steps then 2*NDR DoubleRow steps
    #      (pair-major: one (pair, tc) per step) = NH steps total. ----
    windows = [(g, oc, p) for g in range(2) for oc in range(OC)
               for p in range(2)]

    # extra ops (half-B prologue) interleaved into half-A windows, per h step
    extras = {}                      # (widx, h) -> list of callables
    for h in range(NH - 2):          # window 0: in_scale A two steps ahead
        extras.setdefault((0, h), []).append(lambda h=h: in_scale(0, h + 2))
    for h2 in range(NH):             # router B fills window-0 JIT stalls
        # (x-B pair h2//2 lands ~2 steps before step h2+5); overflow -> w1
        w, s = (0, h2 + 5) if h2 + 5 < NH else (1, 2 * (h2 + 5 - NH))
        extras.setdefault((w, s), []).append(lambda h2=h2: router_mm(1, h2))
    extras.setdefault((1, 12), []).append(lambda: softmax_a(1))
    extras.setdefault((1, 24), []).append(lambda: softmax_b(1))
    for h2 in range(NH):             # windows 2-12: in_scale B (3 / window)
        w = 2 + h2 // 3
        extras.setdefault((w, (h2 % 3) * 10 + 5), []).append(
            lambda h2=h2: in_scale(1, h2))

    def fire_extras(widx, step):
        for fn in extras.pop((widx, step), ()):
            fn()

    prev = None
    for widx, (gen, oc, p) in enumerate(windows):
        tcs = [gen * 4 + p * 2, gen * 4 + p * 2 + 1]
        last = widx == len(windows) - 1
        if prev is not None:
            for tcc in prev[1]:
                emit_drain(prev[0], prev[2], tcc)
        if widx >= 2:
            emit_os(gen, oc, tcs)
        for tcc in tcs:
            mains[(oc, tcc)] = psum.tile([128, ow], F32,
                                         name=f"mp{gen}_{oc}_{tcc}",
                                         tag="mps", bufs=4)
        ngen, noc = (gen, oc + 1) if oc + 1 < OC else (gen + 1, 0)
        pref = p == 1 and ngen < 2
        if pref:
            dma_bias(noc)
        if last:
            # column-major: finish tc by tc so the tail drain is short
            for tcc in tcs:
                for h in range(NBF):
                    mm_bf(gen, oc, tcc, h)
                for ihp in range(NDR):
                    mm_dr(gen, oc, tcc, ihp)
                if tcc != tcs[1]:
                    emit_drain(gen, oc, tcc)
            # final tc drained in halves so DVE/GpSimd/DMA pipeline
            tcc = tcs[1]
            mp = mains.pop((oc, tcc))
            ot = os_t.pop((oc, tcc))
            for k in range(2):
                sl = slice(k * (ow // 2), (k + 1) * (ow // 2))
                yt = pool.tile([128, ow // 2], F32, name=f"ytf{k}", tag="ytf",
                               bufs=2)
                nc.vector.tensor_tensor(yt[:], mp[:, sl], ot[:, sl], OP.mult)
                yt2 = pool.tile([128, ow // 2], F32, name=f"ytf2{k}",
                                tag="ytf2", bufs=2)
                nc.vector.tensor_tensor(yt2[:], yt[:],
                                        bias_t[oc][:, sl], OP.add)
                nc.scalar.dma_start(y[tcc * 128:(tcc + 1) * 128,
                                      oc * ow + k * (ow // 2):
                                      oc * ow + (k + 1) * (ow // 2)], yt2[:])
            prev = None
            break
        for h in range(NBF):
            fire_extras(widx, h)
            if pref and h % GS == 0:
                dma_w(ngen, noc, h // GS)
            for tcc in tcs:
                mm_bf(gen, oc, tcc, h)
        for step in range(NBF, NH):
            fire_extras(widx, step)
            s = step - NBF
            if pref and s == 0:
                dma_w8(ngen, noc)
            # pair-major: window 0 consumes pair hp at step NBF+s, one step
            # after its in_scale fires (JIT)
            ihp = s // 2
            tcc = tcs[s % 2]
            mm_dr(gen, oc, tcc, ihp)
        prev = (gen, tcs, oc)
    assert not extras, f"unconsumed extras: {list(extras)[:4]}"


import numpy as np
import ml_dtypes

NCORES = 8
B, S, H, O, E = 4, 2048, 4096, 4096, 4
N = B * S
Nc = N // NCORES
NDR = 7
NBF = H // 128 - 2 * NDR
GS = 9
CFG = dict(H=H, O=O, Nc=Nc, E=E, ow=512, NDR=NDR, GS=GS)
BF16_NP = ml_dtypes.bfloat16
FP8_NP = ml_dtypes.float8_e4m3

TRACE = False
LAST_EXEC_NS = None
LAST_TRACE_PATH = None
_NC_CACHE = None


def _get_nc():
    global _NC_CACHE
    if _NC_CACHE is None:
        import concourse.bacc as bacc
        import concourse.tile as tile
        nc = bacc.Bacc("TRN2", target_bir_lowering=False, debug=False,
                       num_devices=NCORES)
        ins_aps = {
            "xP0": nc.dram_tensor("xP0", [128, (H // 128) * 512], BF16,
                                  kind="ExternalInput").ap(),
            "xP1": nc.dram_tensor("xP1", [128, (H // 128) * 512], BF16,
                                  kind="ExternalInput").ap(),
            "wbP": nc.dram_tensor("wbP", [128, NBF, O], FP8,
                                  kind="ExternalInput").ap(),
            "wdr": nc.dram_tensor("wdr", [128, O // 512, NDR * 2 * 512],
                                  FP8, kind="ExternalInput").ap(),
            "gwP": nc.dram_tensor("gwP", [128, (H // 128) * E], BF16,
                                  kind="ExternalInput").ap(),
            "ics": nc.dram_tensor("ics", [E, H], BF16, kind="ExternalInput").ap(),
            "ocs_f": nc.dram_tensor("ocs_f", [E, O], BF16,
                                    kind="ExternalInput").ap(),
            "bias2": nc.dram_tensor("bias2", [128, O], F32,
                                    kind="ExternalInput").ap(),
            "ones_e": nc.dram_tensor("ones_e", [E, E], BF16,
                                     kind="ExternalInput").ap(),
        }
        outs_aps = {"y": nc.dram_tensor("y", [Nc, O], F32,
                                        kind="ExternalOutput").ap()}
        with tile.TileContext(nc) as tc:
            with ExitStack() as ctx:
                build_moe8(ctx, tc, outs_aps, ins_aps, CFG)
        nc.compile()
        _NC_CACHE = nc
    return _NC_CACHE


def kernel(x, weight, bias, gate_w, in_channel_scale, out_channel_scale):
    """Full inputs in, full output out; distributes over 8 NeuronCores."""
    global LAST_EXEC_NS, LAST_TRACE_PATH
    from concourse.bass_utils import run_bass_kernel_spmd

    x = np.asarray(x, dtype=np.float32)
    weight = np.asarray(weight, dtype=np.float32)
    bias = np.asarray(bias, dtype=np.float32)
    gate_w = np.asarray(gate_w, dtype=np.float32)
    ics = np.asarray(in_channel_scale, dtype=np.float32)
    ocs = np.asarray(out_channel_scale, dtype=np.float32)

    nc = _get_nc()
    xf = x.reshape(N, H)
    wsT = np.sign(weight).T.astype(FP8_NP)          # [H, O]
    # bf16-path weights: [p, h, o] so one DMA grabs GS chunks x 512 ocols
    wbP = np.ascontiguousarray(
        wsT[:NBF * 128, :].reshape(NBF, 128, O).transpose(1, 0, 2))
    # DoubleRow tail: [p, oc, (ihp, plane), (half, c)]
    wdr = np.ascontiguousarray(
        wsT[NBF * 128:, :]
        .reshape(NDR, 2, 128, O // 512, 2, 256)      # ihp plane p oc half c
        .transpose(2, 3, 0, 1, 4, 5)
        .reshape(128, O // 512, NDR * 2 * 512))
    gwP = np.ascontiguousarray(
        gate_w.T.reshape(H // 128, 128, E).transpose(1, 0, 2)
        .reshape(128, (H // 128) * E)).astype(BF16_NP)
    ics_b = ics.astype(BF16_NP)
    ocs_b = ocs.astype(BF16_NP)
    bias2 = np.ascontiguousarray(np.broadcast_to(bias[None, :], (128, O)))
    ones_e = np.ones((E, E), dtype=BF16_NP)
    in_maps = []
    for c in range(NCORES):
        xt_c = xf[c * Nc:(c + 1) * Nc, :].T.reshape(H // 128, 128, Nc)
        xp = [np.ascontiguousarray(
                  xt_c[:, :, hf * 512:(hf + 1) * 512].transpose(1, 0, 2)
                  .reshape(128, (H // 128) * 512)).astype(BF16_NP)
              for hf in range(2)]
        in_maps.append({
            "xP0": xp[0], "xP1": xp[1],
            "wbP": wbP, "wdr": wdr, "gwP": gwP, "ics": ics_b,
            "ocs_f": ocs_b, "bias2": bias2, "ones_e": ones_e,
        })
    res = run_bass_kernel_spmd(nc, in_maps, core_ids=list(range(NCORES)),
                               trace=TRACE)
    if TRACE:
        LAST_EXEC_NS = res.exec_time_ns
        if res.instructions_and_trace:
            LAST_TRACE_PATH = res.instructions_and_trace[1]
    yfull = np.concatenate([res.results[c]["y"] for c in range(NCORES)], axis=0)
    return yfull.reshape(B, S, O)
